# revision 1
# baseline (speedup 1.0000x reference)
"""Trainium2 Bass kernel for AttentionWithRoPE (B=2, S=2048, HID=2048, H=16, D=128).

Sharding (8 cores): tensor-parallel over heads x data-parallel over batch.
Core c handles batch c//4 and heads 4*(c%4) .. 4*(c%4)+4:
  - QKV^T projections as fp32r PE matmuls with the HID contraction on
    partitions (hidden arrives pre-transposed from the host). Q pass, K pass,
    then V pass; Q^T/K^T/V stay resident in SBUF (a 2-slot rotating pool
    hosts wq/wk -> wv -> A^T over time, so each phase's loads prefetch
    during the previous one). RoPE is fused on the DVE reading the
    projection PSUM directly (partition-shifted reads are legal vs PSUM).
  - Causal attention per head in scores^T orientation ([k, q]: the PV
    contraction dim k lands on partitions, so P^T feeds the PE directly and
    softmax needs no transposes). exp on ScalarE straight from PSUM with the
    1/sqrt(D) scale folded in; fully-masked k-blocks are skipped; diagonal
    blocks multiply a 0/1 upper-tri mask; the softmax denominator is a
    ones-vector PE matmul accumulated alongside PV; normalization is
    reciprocal + K=1 PE-matmul broadcast + DVE multiply.
  - Output projection partial with the core's w_o column slice; the host
    sums the four partials per batch (the TP reduce).
All matmul operands are float32r (TF32-like: full PE rate at moving
free-dim >= 256, ~1.5e-4 rel err); end-to-end output error vs the fp32
reference is ~2e-4. Big DMAs are chunked so consumers start on the first
chunk; small DMAs are merged to amortize descriptor cost.
"""
import numpy as np
from contextlib import ExitStack

import concourse.bass as bass
import concourse.tile as tile
from concourse import bacc, mybir
from concourse.bass_utils import run_bass_kernel_spmd

B, S, HID = 2, 2048, 2048
H, D = 16, 128
NCORES = 8
NH = 4                 # heads per core
HC = HID // 128        # hid chunks
ST = 256               # phase-A s-tile width
NST = S // ST
QT = 512               # phase-B q-tile width
NQT = S // QT
DSCALE = float(D) ** -0.5
F32 = mybir.dt.float32
F32R = mybir.dt.float32r

_CACHED = {}


def _build_nc():
    nc = bacc.Bacc("TRN2", target_bir_lowering=False, debug=False,
                   num_devices=NCORES)
    hT = nc.dram_tensor("hT", [HID, S], F32R, kind="ExternalInput")
    wqT = nc.dram_tensor("wqT", [HID, NH * D], F32R, kind="ExternalInput")
    wkT = nc.dram_tensor("wkT", [HID, NH * D], F32R, kind="ExternalInput")
    wvT = nc.dram_tensor("wvT", [HID, NH * D], F32R, kind="ExternalInput")
    woT = nc.dram_tensor("woT", [NH * D, HID], F32R, kind="ExternalInput")
    cosT = nc.dram_tensor("cosT", [D, S], F32, kind="ExternalInput")
    sinS = nc.dram_tensor("sinS", [D, S], F32, kind="ExternalInput")
    tri = nc.dram_tensor("tri", [128, 128], F32, kind="ExternalInput")
    ones = nc.dram_tensor("ones", [128, 1], F32R, kind="ExternalInput")
    onesr = nc.dram_tensor("onesr", [1, 128], F32R, kind="ExternalInput")
    out = nc.dram_tensor("out", [S, HID], F32, kind="ExternalOutput")

    hT_r = hT.ap().rearrange("(hc p) s -> p hc s", p=128)
    wqT_r = wqT.ap().rearrange("(hc p) m -> p hc m", p=128)
    wkT_r = wkT.ap().rearrange("(hc p) m -> p hc m", p=128)
    wvT_r = wvT.ap().rearrange("(hc p) m -> p hc m", p=128)
    woT_r = woT.ap().rearrange("(g p) n -> p g n", p=128)

    AST = 512              # phase-A s-tile width (N of the QK-pass matmuls)
    ANST = S // AST

    with tile.TileContext(nc) as tc, ExitStack() as ctx:
        # ---- small constants ----
        constp = ctx.enter_context(tc.tile_pool(name="const", bufs=1))
        tri_sb = constp.tile([128, 128], F32, tag="tri", name="tri")
        nc.sync.dma_start(tri_sb[:], tri.ap())
        ones_sb = constp.tile([128, 1], F32R, tag="ones", name="ones")
        nc.sync.dma_start(ones_sb[:], ones.ap())
        onesr_sb = constp.tile([1, 128], F32R, tag="onesr", name="onesr")
        nc.sync.dma_start(onesr_sb[:], onesr.ap())

        # Q^T/K^T stay resident in SBUF through attention
        qkp = ctx.enter_context(tc.tile_pool(name="qk", bufs=1))
        qsb = qkp.tile([128, NH, S], F32R, tag="qsb", name="qsb")
        ksb = qkp.tile([128, NH, S], F32R, tag="ksb", name="ksb")

        # Rotating 2-slot pool (32KB/partition each) hosting, over time:
        #   slot0: wq -> wv -> at   slot1: wk -> v_sb
        # Tile's WAR tracking turns each reuse into a prefetch window.
        wpool = ctx.enter_context(tc.tile_pool(name="aw", bufs=2))
        wq_sb = wpool.tile([128, HC, NH * D], F32R, tag="w", name="wq")
        wk_sb = wpool.tile([128, HC, NH * D], F32R, tag="w", name="wk")
        for c in range(4):
            h4 = slice(4 * c, 4 * c + 4)
            nc.sync.dma_start(wq_sb[:, h4, :], wqT_r[:, h4, :])
        for c in range(4):
            h4 = slice(4 * c, 4 * c + 4)
            nc.sync.dma_start(wk_sb[:, h4, :], wkT_r[:, h4, :])

        # ================= Phase A: Q pass, K pass =================
        with ExitStack() as astack:
            hpool = astack.enter_context(tc.tile_pool(name="ah", bufs=3))
            cspool = astack.enter_context(tc.tile_pool(name="acs", bufs=2))
            ropep = astack.enter_context(tc.tile_pool(name="arope", bufs=1))
            psA = astack.enter_context(
                tc.tile_pool(name="apsqk", bufs=5, space="PSUM"))

            for st in range(ANST):
                sl = bass.ts(st, AST)
                hb = [hpool.tile([128, HC // 2, AST], F32R, tag="h",
                                 name=f"hb{half}") for half in range(2)]
                for half in range(2):
                    for c in range(4):
                        nc.sync.dma_start(
                            hb[half][:, 2 * c:2 * c + 2, :],
                            hT_r[:, slice(8 * half + 2 * c,
                                          8 * half + 2 * c + 2), sl])
                cs_t = cspool.tile([128, AST], F32, tag="cs", name="cs")
                nc.sync.dma_start(cs_t[:], cosT.ap()[:, sl])
                ss_t = cspool.tile([128, AST], F32, tag="ss", name="ss")
                nc.sync.dma_start(ss_t[:], sinS.ap()[:, sl])
                for wsb, dsb in ((wq_sb, qsb), (wk_sb, ksb)):
                    for h in range(NH):
                        ps = psA.tile([128, AST], F32, tag="psqk",
                                      name="psqk")
                        for hc in range(HC):
                            nc.tensor.matmul(
                                ps[:],
                                wsb[:, hc, h * D:(h + 1) * D],
                                hb[hc // 8][:, hc % 8, :],
                                start=(hc == 0), stop=(hc == HC - 1),
                            )
                        # RoPE: out = x*cos + shift(x)*sin_signed. The
                        # partition-shifted reads go straight to PSUM (walrus
                        # requires equal base partitions only when BOTH
                        # operands are in SBUF).
                        tsin = ropep.tile([128, AST], F32, tag="tsin",
                                          name="tsin")
                        nc.vector.tensor_tensor(
                            tsin[0:64, :], ps[64:128, :], ss_t[0:64, :],
                            mybir.AluOpType.mult)
                        nc.vector.tensor_tensor(
                            tsin[64:128, :], ps[0:64, :], ss_t[64:128, :],
                            mybir.AluOpType.mult)
                        tcos = ropep.tile([128, AST], F32, tag="tcos",
                                          name="tcos")
                        nc.vector.tensor_tensor(
                            tcos[:], ps[:], cs_t[:], mybir.AluOpType.mult)
                        nc.vector.tensor_tensor(
                            dsb[:, h, sl], tcos[:], tsin[:],
                            mybir.AluOpType.add)

            # wv reuses wq's slot; its loads overlap the tail of the QK pass
            wv_sb = wpool.tile([128, HC, NH * D], F32R, tag="w", name="wv")
            for c in range(4):
                h4 = slice(4 * c, 4 * c + 4)
                nc.sync.dma_start(wv_sb[:, h4, :], wvT_r[:, h4, :])

        # ================= Phase A2: V projection =================
        # v_sb reuses wk's slot; natural orientation, resident through B
        v_sb = wpool.tile([128, S // 128, NH * D], F32R, tag="w", name="vsb")
        with ExitStack() as a2ctx:
            h2pool = a2ctx.enter_context(tc.tile_pool(name="ah2", bufs=4))
            psAv = a2ctx.enter_context(
                tc.tile_pool(name="apsv", bufs=3, space="PSUM"))
            for st in range(NST):
                sl = bass.ts(st, ST)
                hq = [h2pool.tile([128, 4, ST], F32R, tag="h2",
                                  name=f"hq{q}") for q in range(4)]
                for q in range(4):
                    nc.sync.dma_start(hq[q][:], hT_r[:, 4 * q:4 * q + 4, sl])
                for sc in range(ST // 128):
                    ps = psAv.tile([128, NH * D], F32, tag="psv", name="psv")
                    for hc in range(HC):
                        nc.tensor.matmul(
                            ps[:],
                            hq[hc // 4][:, hc % 4, sc * 128:(sc + 1) * 128],
                            wv_sb[:, hc, :],
                            start=(hc == 0), stop=(hc == HC - 1),
                        )
                    nc.scalar.copy(
                        v_sb[:, st * (ST // 128) + sc, :], ps[:])

        # A^T (phase B -> C) reuses wv's slot; w_o prefetches during B
        at_all = wpool.tile([128, NH, S], F32R, tag="w", name="at")
        wop = ctx.enter_context(tc.tile_pool(name="cwo", bufs=1))
        wo_sb = wop.tile([128, NH, HID], F32R, tag="wo", name="wo")
        for g in range(NH):
            nc.sync.dma_start(wo_sb[:, g, :], woT_r[:, g, :])

        # ================= Phase B =================
        with ExitStack() as bctx:
            expp = bctx.enter_context(tc.tile_pool(name="bexp", bufs=6))
            smallp = bctx.enter_context(tc.tile_pool(name="bsmall", bufs=3))
            psS = bctx.enter_context(
                tc.tile_pool(name="bpss", bufs=2, space="PSUM"))
            psPV = bctx.enter_context(
                tc.tile_pool(name="bpspv", bufs=2, space="PSUM"))
            psCS = bctx.enter_context(
                tc.tile_pool(name="bpscs", bufs=2, space="PSUM"))

            for h in range(NH):
                for qt in range(NQT):
                    nallow = (QT // 128) * qt + (QT // 128)
                    qsl = bass.ts(qt, QT)
                    pvps = psPV.tile([128, QT], F32, tag="pv", name="pv")
                    csps = psCS.tile([1, QT], F32, tag="cs", name="cs")

                    # scores^T in 2-chunk PSUM groups; exp to SBUF groups
                    ngrp = (nallow + 1) // 2
                    egrp = []
                    for g in range(ngrp):
                        k0 = 2 * g
                        nk = min(2, nallow - k0)
                        sps = psS.tile([128, 2, QT], F32, tag="s", name="s")
                        eb = expp.tile([128, 2, QT], F32R, tag="e", name="e")
                        egrp.append(eb)
                        for i in range(nk):
                            kc = k0 + i
                            lo = max(0, 128 * (kc - 4 * qt))
                            nc.tensor.matmul(
                                sps[:, i, lo:QT],
                                ksb[:, h, kc * 128:(kc + 1) * 128],
                                qsb[:, h, qt * QT + lo:(qt + 1) * QT],
                                start=True, stop=True,
                            )
                        j0 = k0 - 4 * qt
                        if j0 + nk - 1 < 0:
                            nc.scalar.activation(
                                eb[:, 0:nk, :], sps[:, 0:nk, :],
                                mybir.ActivationFunctionType.Exp,
                                scale=DSCALE)
                        else:
                            for i in range(nk):
                                kc = k0 + i
                                j = kc - 4 * qt
                                lo = max(0, 128 * j)
                                nc.scalar.activation(
                                    eb[:, i, lo:QT], sps[:, i, lo:QT],
                                    mybir.ActivationFunctionType.Exp,
                                    scale=DSCALE)
                                if j >= 0:
                                    nc.vector.tensor_tensor(
                                        eb[:, i, lo:lo + 128],
                                        eb[:, i, lo:lo + 128].bitcast(F32),
                                        tri_sb[:],
                                        mybir.AluOpType.mult)

                    # colsum + PV accumulation over allowed chunks
                    for kc in range(nallow):
                        j = kc - 4 * qt
                        lo = max(0, 128 * j)
                        eb = egrp[kc // 2]
                        i = kc % 2
                        nc.tensor.matmul(
                            csps[:, lo:QT], ones_sb[:],
                            eb[:, i, lo:QT],
                            start=(kc == 0), stop=(kc == nallow - 1),
                            skip_group_check=True,
                        )
                        nc.tensor.matmul(
                            pvps[:, lo:QT],
                            v_sb[:, kc, h * D:(h + 1) * D],
                            eb[:, i, lo:QT],
                            start=(kc == 0), stop=(kc == nallow - 1),
                            skip_group_check=True,
                        )

                    # normalize: at = pv * broadcast(1/colsum). Broadcast
                    # via a K=1 PE matmul (ones column x reciprocal row).
                    rec = smallp.tile([1, QT], F32R, tag="rec", name="rec")
                    with nc.allow_low_precision(
                            reason="softmax denom reciprocal to f32r"):
                        nc.vector.reciprocal(rec[:], csps[:])
                    rbc = psPV.tile([128, QT], F32, tag="pv", name="rbc")
                    nc.tensor.matmul(rbc[:], onesr_sb[:], rec[:],
                                     start=True, stop=True)
                    at_t = smallp.tile([128, QT], F32, tag="att", name="att")
                    nc.vector.tensor_copy(at_t[:], pvps[:])
                    nc.vector.tensor_tensor(
                        at_all[:, h, qsl], at_t[:], rbc[:],
                        mybir.AluOpType.mult)

        # ================= Phase C =================
        with ExitStack() as cctx:
            outp = cctx.enter_context(tc.tile_pool(name="cout", bufs=3))
            psO = cctx.enter_context(
                tc.tile_pool(name="cpso", bufs=4, space="PSUM"))

            for sc in range(S // 128):
                ssl = bass.ts(sc, 128)
                ot = outp.tile([128, HID], F32, tag="ot", name="ot")
                for nt in range(HID // QT):
                    nsl = bass.ts(nt, QT)
                    ps = psO.tile([128, QT], F32, tag="o", name="o")
                    for g in range(NH):
                        nc.tensor.matmul(
                            ps[:],
                            at_all[:, g, ssl],
                            wo_sb[:, g, nsl],
                            start=(g == 0), stop=(g == NH - 1),
                        )
                    if nt % 2 == 0:
                        nc.vector.tensor_copy(ot[:, nsl], ps[:])
                    else:
                        nc.scalar.copy(ot[:, nsl], ps[:])
                nc.sync.dma_start(out.ap()[ssl, :], ot[:])

    nc.compile()
    return nc


def _prep_in_maps(hidden_states, cos, sin, w_qkv, w_o):
    hs = np.ascontiguousarray(np.asarray(hidden_states, dtype=np.float32))
    cos = np.asarray(cos, dtype=np.float32)
    sin = np.asarray(sin, dtype=np.float32)
    w_qkv = np.asarray(w_qkv, dtype=np.float32)
    w_o = np.asarray(w_o, dtype=np.float32)

    wT = np.ascontiguousarray(w_qkv.T)          # (HID, 3*H*D)
    woTf = np.ascontiguousarray(w_o.T)          # (H*D, HID)
    cosT = np.ascontiguousarray(cos.T)          # (D, S)
    sinT = np.ascontiguousarray(sin.T)
    sinS = sinT.copy()
    sinS[:64] = -sinT[:64]
    tri = np.triu(np.ones((128, 128), np.float32))
    ones = np.ones((128, 1), np.float32)

    hT = [np.ascontiguousarray(hs[b].T) for b in range(B)]

    in_maps = []
    for c in range(NCORES):
        b, hg = c // 4, c % 4
        lo, hi = hg * NH * D, (hg + 1) * NH * D
        in_maps.append({
            "hT": hT[b],
            "wqT": np.ascontiguousarray(wT[:, lo:hi]),
            "wkT": np.ascontiguousarray(wT[:, H * D + lo:H * D + hi]),
            "wvT": np.ascontiguousarray(wT[:, 2 * H * D + lo:2 * H * D + hi]),
            "woT": np.ascontiguousarray(woTf[lo:hi, :]),
            "cosT": cosT,
            "sinS": sinS,
            "tri": tri,
            "ones": ones,
            "onesr": np.ones((1, 128), np.float32),
        })
    return in_maps


def kernel(hidden_states, cos, sin, w_qkv, w_o, _trace=False):
    if "nc" not in _CACHED:
        _CACHED["nc"] = _build_nc()
    nc = _CACHED["nc"]
    in_maps = _prep_in_maps(hidden_states, cos, sin, w_qkv, w_o)
    res = run_bass_kernel_spmd(nc, in_maps, core_ids=list(range(NCORES)),
                               trace=_trace)
    _CACHED["last_result"] = res
    out = np.zeros((B, S, HID), np.float32)
    for c in range(NCORES):
        out[c // 4] += res.results[c]["out"]
    return out



# revision 2
# speedup vs baseline: 1.2972x; 1.2972x over previous
"""Trainium2 Bass kernel for AttentionWithRoPE (B=2, S=2048, HID=2048, H=16, D=128).

Sharding (8 cores): tensor-parallel over heads x data-parallel over batch.
Core c handles batch c//4 and heads 4*(c%4) .. 4*(c%4)+4.

Numerics: projections (QKV, V, output) run as hierarchical-fp8 DoubleRow
matmuls — each operand is split on the host into hi = fp8(x*scale) and an
UNSCALED residual lo = fp8(x*scale - hi), and the product keeps the three
large cross terms (hi*hi + lo*hi + hi*lo), dropping lo*lo (~0.1% error).
DoubleRow contracts two 128-deep k-tiles per instruction at 0.5 PE
cycles/row, so the 3-term product costs 1.5 cycles per 256 contraction vs
2.0 for fp32r. Attention (scores/exp/colsum/PV) runs in bf16 (1 cycle/row
at any tile size). End-to-end rel err vs the fp32 reference ~3e-3.

Schedule: V projection shares phase A's hidden tiles (h loaded once);
score chunks use single-chunk PSUM groups with colsum/PV interleaved one
chunk behind so the scalar-engine exp never throttles the PE through PSUM
WAR dependencies; softmax normalization is software-pipelined one (h,qt)
iteration late so the reciprocal latency hides under the next iteration's
score matmuls. DMAs are chunk-interleaved so the first matmul starts ~2us
after launch. Output partials are written bf16 and summed on the host
(the TP reduce).
"""
import numpy as np
import ml_dtypes
from contextlib import ExitStack

import concourse.bass as bass
import concourse.tile as tile
from concourse import bacc, mybir
from concourse.bass_utils import run_bass_kernel_spmd

B, S, HID = 2, 2048, 2048
H, D = 16, 128
NCORES = 8
NH = 4                 # heads per core
HC = HID // 128        # hid chunks
NP = HC // 2           # DoubleRow chunk pairs
AST = 512              # phase-A s-tile width
ANST = S // AST
QT = 512               # phase-B q-tile width
NQT = S // QT
DSCALE = float(D) ** -0.5
SH, SW, SA = 16.0, 1024.0, 16.0   # fp8 scales: hidden, weights, attn-out
F32 = mybir.dt.float32
F32R = mybir.dt.float32r
BF16 = mybir.dt.bfloat16
F8 = mybir.dt.float8e4
NF8 = ml_dtypes.float8_e4m3
NBF = ml_dtypes.bfloat16
DR = mybir.MatmulPerfMode.DoubleRow

_CACHED = {}


def _build_nc():
    nc = bacc.Bacc("TRN2", target_bir_lowering=False, debug=False,
                   num_devices=NCORES)
    hT_hi = nc.dram_tensor("hT_hi", [HID, S], F8, kind="ExternalInput")
    hT_lo = nc.dram_tensor("hT_lo", [HID, S], F8, kind="ExternalInput")
    w_in = {}
    for w in ("wq", "wk", "wv"):
        for p in ("hi", "lo"):
            w_in[f"{w}_{p}"] = nc.dram_tensor(
                f"{w}_{p}", [HID, NH * D], F8, kind="ExternalInput")
    wo_hi_d = nc.dram_tensor("wo_hi", [NH * D, HID], F8, kind="ExternalInput")
    wo_lo_d = nc.dram_tensor("wo_lo", [NH * D, HID], F8, kind="ExternalInput")
    cosT = nc.dram_tensor("cosT", [D, S], F32, kind="ExternalInput")
    sinS = nc.dram_tensor("sinS", [D, S], F32, kind="ExternalInput")
    tri = nc.dram_tensor("tri", [128, 128], BF16, kind="ExternalInput")
    ones = nc.dram_tensor("ones", [128, 1], BF16, kind="ExternalInput")
    onesr = nc.dram_tensor("onesr", [1, 128], F32R, kind="ExternalInput")
    out = nc.dram_tensor("out", [S, HID], BF16, kind="ExternalOutput")

    hhi_r = hT_hi.ap().rearrange("(hc p) s -> p hc s", p=128)
    hlo_r = hT_lo.ap().rearrange("(hc p) s -> p hc s", p=128)
    w_r = {k: v.ap().rearrange("(hc p) m -> p hc m", p=128)
           for k, v in w_in.items()}
    wohi_r = wo_hi_d.ap().rearrange("(g p) n -> p g n", p=128)
    wolo_r = wo_lo_d.ap().rearrange("(g p) n -> p g n", p=128)

    with tile.TileContext(nc) as tc, ExitStack() as ctx:
        # ---- small constants ----
        constp = ctx.enter_context(tc.tile_pool(name="const", bufs=1))
        tri_sb = constp.tile([128, 128], BF16, tag="tri", name="tri")
        nc.sync.dma_start(tri_sb[:], tri.ap())
        ones_sb = constp.tile([128, 1], BF16, tag="ones", name="ones")
        nc.sync.dma_start(ones_sb[:], ones.ap())
        onesr_sb = constp.tile([1, 128], F32R, tag="onesr", name="onesr")
        nc.sync.dma_start(onesr_sb[:], onesr.ap())

        # resident weights (fp8 hi/lo)
        wpool = ctx.enter_context(tc.tile_pool(name="w", bufs=1))
        w_sb = {k: wpool.tile([128, HC, NH * D], F8, tag=k, name=k)
                for k in w_in}
        wop = ctx.enter_context(tc.tile_pool(name="wo", bufs=1))
        wo_hi = wop.tile([128, NH, HID], F8, tag="wohi", name="wohi")
        wo_lo = wop.tile([128, NH, HID], F8, tag="wolo", name="wolo")

        # Q^T/K^T (bf16) resident through attention; V natural orientation
        qkp = ctx.enter_context(tc.tile_pool(name="qk", bufs=1))
        qsb = qkp.tile([128, NH, S], BF16, tag="qsb", name="qsb")
        ksb = qkp.tile([128, NH, S], BF16, tag="ksb", name="ksb")
        v_sb = qkp.tile([128, S // 128, NH * D], BF16, tag="vsb", name="vsb")
        at_hi = qkp.tile([128, NH, S], F8, tag="athi", name="athi")
        at_lo = qkp.tile([128, NH, S], F8, tag="atlo", name="atlo")

        hpool = ctx.enter_context(tc.tile_pool(name="ah", bufs=2))
        cspool = ctx.enter_context(tc.tile_pool(name="acs", bufs=2))

        def load_htile(st):
            sl = bass.ts(st, AST)
            hb_hi = hpool.tile([128, HC, AST], F8, tag="hhi", name="hhi")
            hb_lo = hpool.tile([128, HC, AST], F8, tag="hlo", name="hlo")
            for c in range(4):
                h4 = slice(4 * c, 4 * c + 4)
                nc.sync.dma_start(hb_hi[:, h4, :], hhi_r[:, h4, sl])
            cs_t = cspool.tile([128, AST], F32, tag="cs", name="cs")
            nc.sync.dma_start(cs_t[:], cosT.ap()[:, sl])
            ss_t = cspool.tile([128, AST], F32, tag="ss", name="ss")
            nc.sync.dma_start(ss_t[:], sinS.ap()[:, sl])
            for c in range(4):
                h4 = slice(4 * c, 4 * c + 4)
                nc.sync.dma_start(hb_lo[:, h4, :], hlo_r[:, h4, sl])
            return hb_hi, hb_lo, cs_t, ss_t

        # start-critical DMA order: wq_hi & first h tile first, chunk-
        # interleaved, then the rest of the weights.
        for c in range(4):
            h4 = slice(4 * c, 4 * c + 4)
            nc.sync.dma_start(w_sb["wq_hi"][:, h4, :], w_r["wq_hi"][:, h4, :])
        htile0 = load_htile(0)
        for name in ("wq_lo", "wk_hi", "wk_lo", "wv_hi", "wv_lo"):
            for c in range(4):
                h4 = slice(4 * c, 4 * c + 4)
                nc.sync.dma_start(w_sb[name][:, h4, :], w_r[name][:, h4, :])
        for g in range(NH):
            nc.sync.dma_start(wo_hi[:, g, :], wohi_r[:, g, :])
            nc.sync.dma_start(wo_lo[:, g, :], wolo_r[:, g, :])

        # ================= Phase A: QKV projections + RoPE ============
        with ExitStack() as astack:
            ropep = astack.enter_context(tc.tile_pool(name="arope", bufs=1))
            psA = astack.enter_context(
                tc.tile_pool(name="apsqk", bufs=6, space="PSUM"))

            for st in range(ANST):
                sl = bass.ts(st, AST)
                hb_hi, hb_lo, cs_t, ss_t = (htile0 if st == 0
                                            else load_htile(st))
                for whi, wlo, dsb in (
                        (w_sb["wq_hi"], w_sb["wq_lo"], qsb),
                        (w_sb["wk_hi"], w_sb["wk_lo"], ksb)):
                    # all heads' T1 first so the start only waits on the
                    # hi-part DMAs
                    pss = [psA.tile([128, AST], F32, tag="psqk", name="psqk")
                           for _ in range(NH)]
                    for h in range(NH):
                        hD = slice(h * D, (h + 1) * D)
                        for j in range(NP):
                            jp = slice(2 * j, 2 * j + 2)
                            nc.tensor.matmul(
                                pss[h][:], whi[:, jp, hD], hb_hi[:, jp, :],
                                start=(j == 0), stop=False, perf_mode=DR,
                                skip_group_check=True)
                    for h in range(NH):
                        hD = slice(h * D, (h + 1) * D)
                        for j in range(NP):
                            jp = slice(2 * j, 2 * j + 2)
                            nc.tensor.matmul(
                                pss[h][:], wlo[:, jp, hD], hb_hi[:, jp, :],
                                start=False, stop=False, perf_mode=DR,
                                skip_group_check=True)
                    for h in range(NH):
                        hD = slice(h * D, (h + 1) * D)
                        for j in range(NP):
                            jp = slice(2 * j, 2 * j + 2)
                            nc.tensor.matmul(
                                pss[h][:], whi[:, jp, hD], hb_lo[:, jp, :],
                                start=False, stop=(j == NP - 1), perf_mode=DR,
                                skip_group_check=True)
                        # RoPE fused on DVE reading projection PSUM
                        # (cos/sin arrive pre-scaled by 1/(SH*SW)).
                        ps = pss[h]
                        tsin = ropep.tile([128, AST], F32, tag="tsin",
                                          name="tsin")
                        nc.vector.tensor_tensor(
                            tsin[0:64, :], ps[64:128, :], ss_t[0:64, :],
                            mybir.AluOpType.mult)
                        nc.vector.tensor_tensor(
                            tsin[64:128, :], ps[0:64, :], ss_t[64:128, :],
                            mybir.AluOpType.mult)
                        tcos = ropep.tile([128, AST], F32, tag="tcos",
                                          name="tcos")
                        nc.vector.tensor_tensor(
                            tcos[:], ps[:], cs_t[:], mybir.AluOpType.mult)
                        with nc.allow_low_precision(reason="bf16 q/k"):
                            nc.vector.tensor_tensor(
                                dsb[:, h, sl], tcos[:], tsin[:],
                                mybir.AluOpType.add)

                # V projection reuses the same h tiles (natural orientation)
                for sc in range(AST // 128):
                    ssl = slice(sc * 128, (sc + 1) * 128)
                    ps = psA.tile([128, NH * D], F32, tag="psqk", name="psv")
                    for j in range(NP):
                        jp = slice(2 * j, 2 * j + 2)
                        nc.tensor.matmul(
                            ps[:], hb_hi[:, jp, ssl], w_sb["wv_hi"][:, jp, :],
                            start=(j == 0), stop=False, perf_mode=DR,
                            skip_group_check=True)
                    for j in range(NP):
                        jp = slice(2 * j, 2 * j + 2)
                        nc.tensor.matmul(
                            ps[:], hb_lo[:, jp, ssl], w_sb["wv_hi"][:, jp, :],
                            start=False, stop=False, perf_mode=DR,
                            skip_group_check=True)
                    for j in range(NP):
                        jp = slice(2 * j, 2 * j + 2)
                        nc.tensor.matmul(
                            ps[:], hb_hi[:, jp, ssl], w_sb["wv_lo"][:, jp, :],
                            start=False, stop=(j == NP - 1), perf_mode=DR,
                            skip_group_check=True)
                    with nc.allow_low_precision(reason="bf16 v"):
                        nc.scalar.mul(v_sb[:, st * (AST // 128) + sc, :],
                                      ps[:], 1.0 / (SH * SW))

        # ================= Phase B: attention =================
        with ExitStack() as bctx:
            expp = bctx.enter_context(tc.tile_pool(name="bexp", bufs=6))
            smallp = bctx.enter_context(tc.tile_pool(name="bsmall", bufs=3))
            psS = bctx.enter_context(
                tc.tile_pool(name="bpss", bufs=3, space="PSUM"))
            psPV = bctx.enter_context(
                tc.tile_pool(name="bpspv", bufs=2, space="PSUM"))
            psCS = bctx.enter_context(
                tc.tile_pool(name="bpscs", bufs=2, space="PSUM"))
            psRB = bctx.enter_context(
                tc.tile_pool(name="bpsrb", bufs=1, space="PSUM"))

            def emit_normalize(pend):
                h, qsl, pvps, csps = pend
                rec = smallp.tile([1, QT], F32R, tag="rec", name="rec")
                with nc.allow_low_precision(
                        reason="softmax denom reciprocal to f32r"):
                    nc.vector.reciprocal(rec[:], csps[:])
                # broadcast SA/den to 128 partitions via K=1 PE matmul
                rbc = psRB.tile([128, QT], F32, tag="rbc", name="rbc")
                nc.tensor.matmul(rbc[:], onesr_sb[:], rec[:],
                                 start=True, stop=True)
                at_t = smallp.tile([128, QT], F32, tag="att", name="att")
                nc.scalar.copy(at_t[:], pvps[:])
                tmp = smallp.tile([128, QT], F32, tag="tmp", name="tmp")
                nc.vector.tensor_tensor(tmp[:], at_t[:], rbc[:],
                                        mybir.AluOpType.mult)
                with nc.allow_low_precision(reason="fp8 attn split"):
                    nc.scalar.copy(at_hi[:, h, qsl], tmp[:])
                    nc.vector.tensor_tensor(
                        at_lo[:, h, qsl], tmp[:],
                        at_hi[:, h, qsl], mybir.AluOpType.subtract)

            pending = None
            for h in range(NH):
                for qt in range(NQT):
                    nallow = (QT // 128) * qt + (QT // 128)
                    qsl = bass.ts(qt, QT)
                    pvps = psPV.tile([128, QT], F32, tag="pv", name="pv")
                    csps = psCS.tile([1, QT], F32, tag="cs", name="cs")

                    ebs = []

                    def emit_cspv(i):
                        kc, lo, eb = ebs[i]
                        nc.tensor.matmul(
                            csps[:, lo:QT], ones_sb[:], eb[:, lo:QT],
                            start=(kc == 0), stop=(kc == nallow - 1),
                            skip_group_check=True)
                        nc.tensor.matmul(
                            pvps[:, lo:QT],
                            v_sb[:, kc, h * D:(h + 1) * D], eb[:, lo:QT],
                            start=(kc == 0), stop=(kc == nallow - 1),
                            skip_group_check=True)

                    for kc in range(nallow):
                        j = kc - (QT // 128) * qt
                        lo = max(0, 128 * j)
                        sps = psS.tile([128, QT], F32, tag="s", name="s")
                        nc.tensor.matmul(
                            sps[:, lo:QT],
                            ksb[:, h, kc * 128:(kc + 1) * 128],
                            qsb[:, h, qt * QT + lo:(qt + 1) * QT],
                            start=True, stop=True)
                        eb = expp.tile([128, QT], BF16, tag="e", name="e")
                        nc.scalar.activation(
                            eb[:, lo:QT], sps[:, lo:QT],
                            mybir.ActivationFunctionType.Exp, scale=DSCALE)
                        if j >= 0:
                            with nc.allow_low_precision(reason="bf16 mask"):
                                nc.vector.tensor_tensor(
                                    eb[:, lo:lo + 128],
                                    eb[:, lo:lo + 128].bitcast(BF16),
                                    tri_sb[:], mybir.AluOpType.mult)
                        ebs.append((kc, lo, eb))
                        if kc >= 1:
                            emit_cspv(kc - 1)
                        if kc == 1 and pending is not None:
                            emit_normalize(pending)
                            pending = None
                    emit_cspv(nallow - 1)
                    pending = (h, qsl, pvps, csps)
            emit_normalize(pending)

        # ================= Phase C: output projection =================
        with ExitStack() as cctx:
            outp = cctx.enter_context(tc.tile_pool(name="cout", bufs=3))
            psO = cctx.enter_context(
                tc.tile_pool(name="cpso", bufs=4, space="PSUM"))

            for sc in range(S // 128):
                ssl = bass.ts(sc, 128)
                ot = outp.tile([128, HID], BF16, tag="ot", name="ot")
                for nt in range(HID // QT):
                    nsl = bass.ts(nt, QT)
                    ps = psO.tile([128, QT], F32, tag="o", name="o")
                    for gp in range(NH // 2):
                        g2 = slice(2 * gp, 2 * gp + 2)
                        nc.tensor.matmul(
                            ps[:], at_hi[:, g2, ssl], wo_hi[:, g2, nsl],
                            start=(gp == 0), stop=False, perf_mode=DR,
                            skip_group_check=True)
                    for gp in range(NH // 2):
                        g2 = slice(2 * gp, 2 * gp + 2)
                        nc.tensor.matmul(
                            ps[:], at_lo[:, g2, ssl], wo_hi[:, g2, nsl],
                            start=False, stop=False, perf_mode=DR,
                            skip_group_check=True)
                    for gp in range(NH // 2):
                        g2 = slice(2 * gp, 2 * gp + 2)
                        nc.tensor.matmul(
                            ps[:], at_hi[:, g2, ssl], wo_lo[:, g2, nsl],
                            start=False, stop=(gp == NH // 2 - 1),
                            perf_mode=DR, skip_group_check=True)
                    with nc.allow_low_precision(reason="bf16 out"):
                        if nt % 2 == 0:
                            nc.vector.tensor_scalar(
                                ot[:, nsl], ps[:], 1.0 / (SA * SW), None,
                                mybir.AluOpType.mult)
                        else:
                            nc.scalar.mul(ot[:, nsl], ps[:], 1.0 / (SA * SW))
                nc.sync.dma_start(out.ap()[ssl, :], ot[:])

    nc.compile()
    return nc


def _split8(x, scale):
    xs = x * scale
    hi = np.asarray(xs, dtype=NF8)
    lo = np.asarray(xs - hi.astype(np.float32), dtype=NF8)
    return hi, lo


def _prep_in_maps(hidden_states, cos, sin, w_qkv, w_o):
    hs = np.asarray(hidden_states, dtype=np.float32)
    cos = np.asarray(cos, dtype=np.float32)
    sin = np.asarray(sin, dtype=np.float32)
    w_qkv = np.asarray(w_qkv, dtype=np.float32)
    w_o = np.asarray(w_o, dtype=np.float32)

    wT = np.ascontiguousarray(w_qkv.T)          # (HID, 3*H*D)
    woTf = np.ascontiguousarray(w_o.T)          # (H*D, HID)
    cosT = np.ascontiguousarray(cos.T) / (SH * SW)
    sinT = np.ascontiguousarray(sin.T) / (SH * SW)
    sinS = sinT.copy()
    sinS[:64] = -sinT[:64]
    tri = np.asarray(np.triu(np.ones((128, 128), np.float32)), dtype=NBF)
    ones = np.asarray(np.ones((128, 1), np.float32), dtype=NBF)
    onesr = np.full((1, 128), SA, np.float32)

    hT = [np.ascontiguousarray(hs[b].T) for b in range(B)]
    hT8 = [_split8(h, SH) for h in hT]

    in_maps = []
    for c in range(NCORES):
        b, hg = c // 4, c % 4
        lo, hi = hg * NH * D, (hg + 1) * NH * D
        wq_hi, wq_lo = _split8(np.ascontiguousarray(wT[:, lo:hi]), SW)
        wk_hi, wk_lo = _split8(
            np.ascontiguousarray(wT[:, H * D + lo:H * D + hi]), SW)
        wv_hi, wv_lo = _split8(
            np.ascontiguousarray(wT[:, 2 * H * D + lo:2 * H * D + hi]), SW)
        wo_hi, wo_lo = _split8(np.ascontiguousarray(woTf[lo:hi, :]), SW)
        in_maps.append({
            "hT_hi": hT8[b][0], "hT_lo": hT8[b][1],
            "wq_hi": wq_hi, "wq_lo": wq_lo,
            "wk_hi": wk_hi, "wk_lo": wk_lo,
            "wv_hi": wv_hi, "wv_lo": wv_lo,
            "wo_hi": wo_hi, "wo_lo": wo_lo,
            "cosT": cosT, "sinS": sinS,
            "tri": tri, "ones": ones, "onesr": onesr,
        })
    return in_maps


def kernel(hidden_states, cos, sin, w_qkv, w_o, _trace=False):
    if "nc" not in _CACHED:
        _CACHED["nc"] = _build_nc()
    nc = _CACHED["nc"]
    in_maps = _prep_in_maps(hidden_states, cos, sin, w_qkv, w_o)
    res = run_bass_kernel_spmd(nc, in_maps, core_ids=list(range(NCORES)),
                               trace=_trace)
    _CACHED["last_result"] = res
    out = np.zeros((B, S, HID), np.float32)
    for c in range(NCORES):
        out[c // 4] += res.results[c]["out"].astype(np.float32)
    return out
